# revision 16
# baseline (speedup 1.0000x reference)
"""Trainium2 Bass kernel for: ConvTranspose2d(128->256, k=4, s=2, p=1)
-> MaxPool2d(2,2) -> Hardtanh -> spatial mean -> Tanh.

Algebraic restructuring (same as bf16 baseline): the stride-2 transposed
conv decomposes into 4 polyphase 2x2 convolutions whose outputs at pooled
position (i, j) are exactly the 4 elements of the 2x2 maxpool window, so
the 128x128 map is never materialized; everything stays at 64x64.

This version runs the matmuls in fp8 (e4m3) with perf_mode=DoubleRow,
which packs the 2 horizontal taps of each phase into one PE pass
(contraction 256 = 2 taps x 128 cin) at 2 fp8 MACs/cell/cycle - about
2x the bf16 matmul rate.  The moving operand is a triple of horizontally
pre-shifted image planes (dj = -1, 0, +1) in SBUF, each plane stored
densely [66 rows x 64 cols] so an 8-row window is one contiguous
512-element run; a DoubleRow rhs is then [128, 2 planes, 512] with the
plane pair selected by the phase's horizontal parity.

Epilogue works in the negated domain q = T - (p + bias)  (weights are
negated on the host, T = 256 in raw units) so that
    clip(p + bias, -1, 1) = 1 - F/256,  F = min(max(min_phases q, 0), 512),
which maps onto cheap fused engine ops with the phase-min tree split
across ACT (Relu evac = max(q,0), distributes through min), DVE
(scalar_tensor_tensor fused add+min, and the final clip+sum via
scalar_tensor_tensor with a const-512 tensor and accum_out), and GPSIMD
(one bf16 tensor_tensor min per block), keeping every engine below the
PE's fill time.  The spatial mean and output tanh fold into one final
activation: out = tanh(1 - sum(F) * 2^-20).

Sharding: data-parallel over batch, 8 images per core on 8 cores,
weights replicated.  fp8 quantization error for this problem measured
3.7e-3 rel (threshold 2e-2): inputs scaled x4, weights x64 to sit in the
e4m3 normal range; all scales fold into the final activation constants.
"""

from contextlib import ExitStack

import ml_dtypes
import numpy as np

import concourse.bacc as bacc
import concourse.bass as bass
import concourse.mybir as mybir
import concourse.tile as tile
from concourse.bass_utils import run_bass_kernel_spmd

# Problem dims (hardcoded per contract)
B, CIN, COUT, H, W = 64, 128, 256, 64, 64
NCORES = 8
BPC = B // NCORES  # images per core

NROW = 66            # padded rows (1 + 64 + 1)
PLANE = NROW * 64    # one shifted plane: 66 rows x 64 valid cols
SX, SW = 4.0, 64.0   # fp8 input/weight scales
SRAW = SX * SW       # raw-unit scale of the conv output (= T)
T2 = 2.0 * SRAW      # upper clip in raw units (512)

F32 = mybir.dt.float32
BF16 = mybir.dt.bfloat16
FP8 = mybir.dt.float8e4


def _tap(ph: int, a: int):
    """For phase parity ph (0=even output coord, 1=odd) and tap index a,
    return (input shift, kernel index) in one dimension.

    ConvTranspose2d(stride=2, pad=1): out[2q+r] = sum over taps of
    x[q+di] * w[k].  r=0: (di,k) in {(0,1), (-1,3)}; r=1: {(1,0), (0,2)}.
    """
    if ph == 0:
        return (0, 1) if a == 0 else (-1, 3)
    return (1, 0) if a == 0 else (0, 2)


def _kw_for_dj(pw: int, dj: int) -> int:
    """Kernel x-index whose input shift equals dj for horizontal parity pw."""
    for b in range(2):
        d, k = _tap(pw, b)
        if d == dj:
            return k
    raise AssertionError


def _wcol(half: int, p: int, va: int) -> int:
    return ((half * 4 + p) * 2 + va) * 256


def build_nc(repeat: int = 1) -> bass.Bass:
    """repeat>1 wraps the compute in a hardware loop executing it `repeat`
    times - used only for wall-clock timing (amortizes the ~80ms axon RPC
    overhead); the graded path uses repeat=1 (no loop)."""
    nc = bacc.Bacc("TRN2", target_bir_lowering=False, debug=False)

    xc = nc.dram_tensor("xc", [BPC, 128, 3 * PLANE], FP8, kind="ExternalInput")
    wm = nc.dram_tensor("wm", [128, 16 * 256], FP8, kind="ExternalInput")
    br = nc.dram_tensor("br", [128, 2], F32, kind="ExternalInput")
    out = nc.dram_tensor("out", [128, 2 * BPC], F32, kind="ExternalOutput")

    Relu = mybir.ActivationFunctionType.Relu
    Tanh = mybir.ActivationFunctionType.Tanh
    ADD = mybir.AluOpType.add
    MIN = mybir.AluOpType.min
    DR = mybir.MatmulPerfMode.DoubleRow

    with ExitStack() as ctx:
        tc = ctx.enter_context(tile.TileContext(nc))
        consts = ctx.enter_context(tc.tile_pool(name="consts", bufs=1))
        canvp = ctx.enter_context(tc.tile_pool(name="canv", bufs=3))
        psump = ctx.enter_context(tc.tile_pool(name="ps", bufs=4, space="PSUM"))
        evp = ctx.enter_context(tc.tile_pool(name="ev", bufs=6))
        mp = ctx.enter_context(tc.tile_pool(name="mt", bufs=9))
        junkp = ctx.enter_context(tc.tile_pool(name="junk", bufs=3))

        w_sb = consts.tile([128, 16 * 256], FP8, tag="w")
        nc.sync.dma_start(w_sb[:], wm[:, :])
        tb_sb = consts.tile([128, 2], F32, tag="tb")
        nc.sync.dma_start(tb_sb[:], br[:, :])
        s_all = consts.tile([128, 2 * BPC * 4], F32, tag="sums")
        s16 = consts.tile([128, 2 * BPC], F32, tag="s16")
        o_sb = consts.tile([128, 2 * BPC], F32, tag="out")

        def body():
            pend = []  # delayed final relu+sum (1-block software pipelining)

            def flush_pend():
                while pend:
                    mg_t, col = pend.pop()
                    junk = junkp.tile([128, 1024], BF16, tag="junk")
                    nc.scalar.activation(
                        junk[:],
                        mg_t[:],
                        Relu,
                        accum_out=s_all[:, col : col + 1],
                    )

            for img in range(BPC):
                canv = canvp.tile([128, 3 * PLANE], FP8, tag="canv")
                nc.sync.dma_start(canv[:], xc[img])
                cv = canv[:].rearrange("p (t n) -> p t n", t=3)
                for half in range(2):
                    tb = tb_sb[:, half : half + 1]
                    for blk in range(8):
                        # psA holds the even phases (0, 2), psB the odd
                        # phases (1, 3): one 1024-wide ACT evac covers both
                        # even phases, one 1024-wide DVE combine covers both
                        # odd ones.
                        psA = psump.tile([128, 2, 512], F32, tag="ps")
                        psB = psump.tile([128, 2, 512], F32, tag="ps")
                        for p in range(4):
                            ph, pw = p >> 1, p & 1
                            ps, slot = (psA, psB)[p & 1], p >> 1
                            for va in range(2):
                                di, _kh = _tap(ph, va)
                                col = _wcol(half, p, va)
                                lhsT = w_sb[:, col : col + 256].rearrange(
                                    "q (two m) -> q two m", two=2
                                )
                                r0 = 1 + 8 * blk
                                off = (r0 + di) * 64
                                nc.tensor.matmul(
                                    ps[:, slot, :],
                                    lhsT,
                                    cv[:, pw : pw + 2, off : off + 512],
                                    start=(va == 0),
                                    stop=(va == 1),
                                    perf_mode=DR,
                                    skip_group_check=True,
                                )
                        # drain: phase-min tree in the negated domain
                        ev = evp.tile([128, 1024], BF16, tag="ev")
                        nc.scalar.activation(
                            ev[:], psA[:].rearrange("q a b -> q (a b)"),
                            Relu, bias=tb,
                        )
                        mA = mp.tile([128, 1024], BF16, tag="m")
                        nc.vector.scalar_tensor_tensor(
                            out=mA[:], in0=psB[:].rearrange("q a b -> q (a b)"),
                            scalar=tb, in1=ev[:], op0=ADD, op1=MIN,
                        )
                        # mgc = min(min(m02, 512), m13): the 512 cap folds
                        # into the combine, written into a 2-block pair tile
                        # so the final relu+sum on ACT runs at 1024 width.
                        if blk % 2 == 0:
                            flush_pend()
                            mgp = mp.tile([128, 1024], BF16, tag="mgp")
                        hof = (blk % 2) * 512
                        nc.vector.scalar_tensor_tensor(
                            out=mgp[:, hof : hof + 512], in0=mA[:, 0:512],
                            scalar=T2, in1=mA[:, 512:1024], op0=MIN, op1=MIN,
                        )
                        if blk % 2 == 1:
                            pend.append((mgp, (img * 2 + half) * 4 + blk // 2))
            flush_pend()
            sv = s_all[:].rearrange("p (u b) -> p u b", b=4)
            nc.vector.tensor_reduce(
                s16[:], sv, axis=mybir.AxisListType.X, op=ADD
            )

        if repeat > 1:
            with tc.For_i(0, repeat, 1):
                body()
        else:
            body()

        # out = tanh(mean) = tanh(1 - sum(F) / (4096 * 256))
        nc.scalar.activation(
            o_sb[:], s16[:], Tanh, bias=1.0, scale=-1.0 / (4096.0 * SRAW)
        )
        nc.sync.dma_start(out[:, :], o_sb[:])

    nc.finalize()
    return nc


_CACHE: dict = {}


def _get_nc() -> bass.Bass:
    if "nc" not in _CACHE:
        _CACHE["nc"] = build_nc()
    return _CACHE["nc"]


def make_in_maps(x: np.ndarray, weight: np.ndarray, bias: np.ndarray):
    x = np.asarray(x, dtype=np.float32)
    weight = np.asarray(weight, dtype=np.float32)
    bias = np.asarray(bias, dtype=np.float32)

    # 3 horizontally shifted planes (dj = -1, 0, +1), zero padded, fp8.
    # Cast once (|x*SX| << 240 so no clip needed), then assemble the
    # shifted planes with byte copies.
    xq = (x * SX).astype(ml_dtypes.float8_e4m3)  # [B, 128, 64, 64]
    canv = np.zeros((B, 128, 3, NROW, 64), dtype=ml_dtypes.float8_e4m3)
    canv[:, :, 0, 1:65, 1:64] = xq[:, :, :, 0:63]  # dj=-1
    canv[:, :, 1, 1:65, 0:64] = xq                 # dj=0
    canv[:, :, 2, 1:65, 0:63] = xq[:, :, :, 1:64]  # dj=+1
    canv = canv.reshape(B, 128, 3 * PLANE)

    # DoubleRow weight pairs: member i multiplies plane pw+i (dj = pw+i-1).
    wmv = np.zeros((128, 16 * 256), dtype=ml_dtypes.float8_e4m3)
    wneg = np.clip(-weight * SW, -240.0, 240.0)
    for half in range(2):
        for p in range(4):
            ph, pw = p >> 1, p & 1
            for va in range(2):
                _di, kh = _tap(ph, va)
                col = _wcol(half, p, va)
                for i in range(2):
                    kw = _kw_for_dj(pw, pw + i - 1)
                    wmv[:, col + i * 128 : col + (i + 1) * 128] = wneg[
                        :, half * 128 : (half + 1) * 128, kh, kw
                    ].astype(ml_dtypes.float8_e4m3)

    # tb = T - 256*bias, per cout partition, one column per half
    tbv = np.ascontiguousarray(
        (SRAW * (1.0 - bias)).reshape(2, 128).T, dtype=np.float32
    )

    return [
        {"xc": canv[c * BPC : (c + 1) * BPC], "wm": wmv, "br": tbv}
        for c in range(NCORES)
    ]


def assemble_output(results: list) -> np.ndarray:
    outs = []
    for c in range(NCORES):
        o = np.asarray(results[c]["out"])  # [128, 2*BPC]
        o = o.reshape(128, BPC, 2).transpose(1, 2, 0).reshape(BPC, COUT)
        outs.append(o)
    return np.concatenate(outs, 0).reshape(B, COUT, 1, 1).astype(np.float32)


def kernel(x: np.ndarray, weight: np.ndarray, bias: np.ndarray) -> np.ndarray:
    nc = _get_nc()
    in_maps = make_in_maps(x, weight, bias)
    res = run_bass_kernel_spmd(nc, in_maps, core_ids=list(range(NCORES)))
    return assemble_output(res.results)


# revision 17
# speedup vs baseline: 1.0422x; 1.0422x over previous
"""Trainium2 Bass kernel for: ConvTranspose2d(128->256, k=4, s=2, p=1)
-> MaxPool2d(2,2) -> Hardtanh -> spatial mean -> Tanh.

Algebraic restructuring (same as bf16 baseline): the stride-2 transposed
conv decomposes into 4 polyphase 2x2 convolutions whose outputs at pooled
position (i, j) are exactly the 4 elements of the 2x2 maxpool window, so
the 128x128 map is never materialized; everything stays at 64x64.

This version runs the matmuls in fp8 (e4m3) with perf_mode=DoubleRow,
which packs the 2 horizontal taps of each phase into one PE pass
(contraction 256 = 2 taps x 128 cin) at 2 fp8 MACs/cell/cycle - about
2x the bf16 matmul rate.  The moving operand is a triple of horizontally
pre-shifted image planes (dj = -1, 0, +1) in SBUF, each plane stored
densely [66 rows x 64 cols] so an 8-row window is one contiguous
512-element run; a DoubleRow rhs is then [128, 2 planes, 512] with the
plane pair selected by the phase's horizontal parity.

Epilogue works in the negated domain q = T - (p + bias)  (weights are
negated on the host, T = 256 in raw units) so that
    clip(p + bias, -1, 1) = 1 - F/256,  F = min(max(min_phases q, 0), 512),
which maps onto cheap fused engine ops with the phase-min tree split
across ACT (Relu evac = max(q,0), distributes through min), DVE
(scalar_tensor_tensor fused add+min, and the final clip+sum via
scalar_tensor_tensor with a const-512 tensor and accum_out), and GPSIMD
(one bf16 tensor_tensor min per block), keeping every engine below the
PE's fill time.  The spatial mean and output tanh fold into one final
activation: out = tanh(1 - sum(F) * 2^-20).

Sharding: data-parallel over batch, 8 images per core on 8 cores,
weights replicated.  fp8 quantization error for this problem measured
3.7e-3 rel (threshold 2e-2): inputs scaled x4, weights x64 to sit in the
e4m3 normal range; all scales fold into the final activation constants.
"""

from contextlib import ExitStack

import ml_dtypes
import numpy as np

import concourse.bacc as bacc
import concourse.bass as bass
import concourse.mybir as mybir
import concourse.tile as tile
from concourse.bass_utils import run_bass_kernel_spmd

# Problem dims (hardcoded per contract)
B, CIN, COUT, H, W = 64, 128, 256, 64, 64
NCORES = 8
BPC = B // NCORES  # images per core

NROW = 66            # padded rows (1 + 64 + 1)
PLANE = NROW * 64    # one shifted plane: 66 rows x 64 valid cols
SX, SW = 4.0, 64.0   # fp8 input/weight scales
SRAW = SX * SW       # raw-unit scale of the conv output (= T)
T2 = 2.0 * SRAW      # upper clip in raw units (512)

F32 = mybir.dt.float32
BF16 = mybir.dt.bfloat16
FP8 = mybir.dt.float8e4


def _tap(ph: int, a: int):
    """For phase parity ph (0=even output coord, 1=odd) and tap index a,
    return (input shift, kernel index) in one dimension.

    ConvTranspose2d(stride=2, pad=1): out[2q+r] = sum over taps of
    x[q+di] * w[k].  r=0: (di,k) in {(0,1), (-1,3)}; r=1: {(1,0), (0,2)}.
    """
    if ph == 0:
        return (0, 1) if a == 0 else (-1, 3)
    return (1, 0) if a == 0 else (0, 2)


def _kw_for_dj(pw: int, dj: int) -> int:
    """Kernel x-index whose input shift equals dj for horizontal parity pw."""
    for b in range(2):
        d, k = _tap(pw, b)
        if d == dj:
            return k
    raise AssertionError


def _wcol(half: int, p: int, va: int) -> int:
    return ((half * 4 + p) * 2 + va) * 256


def build_nc(repeat: int = 1) -> bass.Bass:
    """repeat>1 wraps the compute in a hardware loop executing it `repeat`
    times - used only for wall-clock timing (amortizes the ~80ms axon RPC
    overhead); the graded path uses repeat=1 (no loop)."""
    nc = bacc.Bacc("TRN2", target_bir_lowering=False, debug=False)

    xc = nc.dram_tensor("xc", [BPC, 128, 3 * PLANE], FP8, kind="ExternalInput")
    wm = nc.dram_tensor("wm", [128, 16 * 256], FP8, kind="ExternalInput")
    br = nc.dram_tensor("br", [128, 2], F32, kind="ExternalInput")
    out = nc.dram_tensor("out", [128, 2 * BPC], F32, kind="ExternalOutput")

    Relu = mybir.ActivationFunctionType.Relu
    Tanh = mybir.ActivationFunctionType.Tanh
    ADD = mybir.AluOpType.add
    MIN = mybir.AluOpType.min
    DR = mybir.MatmulPerfMode.DoubleRow

    with ExitStack() as ctx:
        tc = ctx.enter_context(tile.TileContext(nc))
        consts = ctx.enter_context(tc.tile_pool(name="consts", bufs=1))
        canvp = ctx.enter_context(tc.tile_pool(name="canv", bufs=3))
        psump = ctx.enter_context(tc.tile_pool(name="ps", bufs=4, space="PSUM"))
        evp = ctx.enter_context(tc.tile_pool(name="ev", bufs=6))
        mp = ctx.enter_context(tc.tile_pool(name="mt", bufs=9))
        junkp = ctx.enter_context(tc.tile_pool(name="junk", bufs=3))

        w_sb = consts.tile([128, 16 * 256], FP8, tag="w")
        nc.sync.dma_start(w_sb[:], wm[:, :])
        tb_sb = consts.tile([128, 2], F32, tag="tb")
        nc.sync.dma_start(tb_sb[:], br[:, :])
        s_all = consts.tile([128, 2 * BPC * 4], F32, tag="sums")
        s16 = consts.tile([128, 2 * BPC], F32, tag="s16")
        o_sb = consts.tile([128, 2 * BPC], F32, tag="out")

        def body():
            pend = []  # delayed final relu+sum (1-block software pipelining)

            def flush_pend():
                while pend:
                    mg_t, col = pend.pop()
                    junk = junkp.tile([128, 1024], BF16, tag="junk")
                    nc.scalar.activation(
                        junk[:],
                        mg_t[:],
                        Relu,
                        accum_out=s_all[:, col : col + 1],
                    )

            for img in range(BPC):
                canv = canvp.tile([128, 3 * PLANE], FP8, tag="canv")
                nc.sync.dma_start(canv[:], xc[img])
                cv = canv[:].rearrange("p (t n) -> p t n", t=3)
                for half in range(2):
                    tb = tb_sb[:, half : half + 1]
                    for blk in range(8):
                        # psA holds the even phases (0, 2), psB the odd
                        # phases (1, 3): one 1024-wide ACT evac covers both
                        # even phases, one 1024-wide DVE combine covers both
                        # odd ones.
                        psA = psump.tile([128, 2, 512], F32, tag="ps")
                        psB = psump.tile([128, 2, 512], F32, tag="ps")
                        for p in (0, 2, 1, 3):  # psA phases first: its
                            # drain (ACT ev) starts 4 MMs early, shortening
                            # the release chain on the 2-block PSUM rotation.
                            ph, pw = p >> 1, p & 1
                            ps, slot = (psA, psB)[p & 1], p >> 1
                            for va in range(2):
                                di, _kh = _tap(ph, va)
                                col = _wcol(half, p, va)
                                lhsT = w_sb[:, col : col + 256].rearrange(
                                    "q (two m) -> q two m", two=2
                                )
                                r0 = 1 + 8 * blk
                                off = (r0 + di) * 64
                                nc.tensor.matmul(
                                    ps[:, slot, :],
                                    lhsT,
                                    cv[:, pw : pw + 2, off : off + 512],
                                    start=(va == 0),
                                    stop=(va == 1),
                                    perf_mode=DR,
                                    skip_group_check=True,
                                )
                        # drain: phase-min tree in the negated domain
                        ev = evp.tile([128, 1024], BF16, tag="ev")
                        nc.scalar.activation(
                            ev[:], psA[:].rearrange("q a b -> q (a b)"),
                            Relu, bias=tb,
                        )
                        mA = mp.tile([128, 1024], BF16, tag="m")
                        nc.vector.scalar_tensor_tensor(
                            out=mA[:], in0=psB[:].rearrange("q a b -> q (a b)"),
                            scalar=tb, in1=ev[:], op0=ADD, op1=MIN,
                        )
                        # mgc = min(min(m02, 512), m13): the 512 cap folds
                        # into the combine, written into a 2-block pair tile
                        # so the final relu+sum on ACT runs at 1024 width.
                        if blk % 2 == 0:
                            flush_pend()
                            mgp = mp.tile([128, 1024], BF16, tag="mgp")
                        hof = (blk % 2) * 512
                        nc.vector.scalar_tensor_tensor(
                            out=mgp[:, hof : hof + 512], in0=mA[:, 0:512],
                            scalar=T2, in1=mA[:, 512:1024], op0=MIN, op1=MIN,
                        )
                        if blk % 2 == 1:
                            pend.append((mgp, (img * 2 + half) * 4 + blk // 2))
            flush_pend()
            sv = s_all[:].rearrange("p (u b) -> p u b", b=4)
            nc.vector.tensor_reduce(
                s16[:], sv, axis=mybir.AxisListType.X, op=ADD
            )

        if repeat > 1:
            with tc.For_i(0, repeat, 1):
                body()
        else:
            body()

        # out = tanh(mean) = tanh(1 - sum(F) / (4096 * 256))
        nc.scalar.activation(
            o_sb[:], s16[:], Tanh, bias=1.0, scale=-1.0 / (4096.0 * SRAW)
        )
        nc.sync.dma_start(out[:, :], o_sb[:])

    nc.finalize()
    return nc


_CACHE: dict = {}


def _get_nc() -> bass.Bass:
    if "nc" not in _CACHE:
        _CACHE["nc"] = build_nc()
    return _CACHE["nc"]


def make_in_maps(x: np.ndarray, weight: np.ndarray, bias: np.ndarray):
    x = np.asarray(x, dtype=np.float32)
    weight = np.asarray(weight, dtype=np.float32)
    bias = np.asarray(bias, dtype=np.float32)

    # 3 horizontally shifted planes (dj = -1, 0, +1), zero padded, fp8.
    # Cast once (|x*SX| << 240 so no clip needed), then assemble the
    # shifted planes with byte copies.
    xq = (x * SX).astype(ml_dtypes.float8_e4m3)  # [B, 128, 64, 64]
    canv = np.zeros((B, 128, 3, NROW, 64), dtype=ml_dtypes.float8_e4m3)
    canv[:, :, 0, 1:65, 1:64] = xq[:, :, :, 0:63]  # dj=-1
    canv[:, :, 1, 1:65, 0:64] = xq                 # dj=0
    canv[:, :, 2, 1:65, 0:63] = xq[:, :, :, 1:64]  # dj=+1
    canv = canv.reshape(B, 128, 3 * PLANE)

    # DoubleRow weight pairs: member i multiplies plane pw+i (dj = pw+i-1).
    wmv = np.zeros((128, 16 * 256), dtype=ml_dtypes.float8_e4m3)
    wneg = np.clip(-weight * SW, -240.0, 240.0)
    for half in range(2):
        for p in range(4):
            ph, pw = p >> 1, p & 1
            for va in range(2):
                _di, kh = _tap(ph, va)
                col = _wcol(half, p, va)
                for i in range(2):
                    kw = _kw_for_dj(pw, pw + i - 1)
                    wmv[:, col + i * 128 : col + (i + 1) * 128] = wneg[
                        :, half * 128 : (half + 1) * 128, kh, kw
                    ].astype(ml_dtypes.float8_e4m3)

    # tb = T - 256*bias, per cout partition, one column per half
    tbv = np.ascontiguousarray(
        (SRAW * (1.0 - bias)).reshape(2, 128).T, dtype=np.float32
    )

    return [
        {"xc": canv[c * BPC : (c + 1) * BPC], "wm": wmv, "br": tbv}
        for c in range(NCORES)
    ]


def assemble_output(results: list) -> np.ndarray:
    outs = []
    for c in range(NCORES):
        o = np.asarray(results[c]["out"])  # [128, 2*BPC]
        o = o.reshape(128, BPC, 2).transpose(1, 2, 0).reshape(BPC, COUT)
        outs.append(o)
    return np.concatenate(outs, 0).reshape(B, COUT, 1, 1).astype(np.float32)


def kernel(x: np.ndarray, weight: np.ndarray, bias: np.ndarray) -> np.ndarray:
    nc = _get_nc()
    in_maps = make_in_maps(x, weight, bias)
    res = run_bass_kernel_spmd(nc, in_maps, core_ids=list(range(NCORES)))
    return assemble_output(res.results)
